# revision 1
# baseline (speedup 1.0000x reference)
"""Trainium2 Bass kernel for PointNet++-style ball query (nn_BallQuery).

Problem: query [4, 2048, 3] f32, key [4, 8192, 3] f32 -> out [4, 2048, 64] int32.
For each query point, the indices of the first 64 key points (in key order)
with squared distance < 0.1^2; empty slots padded with the first neighbor
index (0 if none).

Sharding (8 NeuronCores): data-parallel over batch B=4 (2 cores per batch),
queries split in halves of 1024 per core; keys of the batch replicated.

Per-core pipeline (8 tiles of 128 queries x 8192 keys, scatters paired):
  PE   : psum = |k|^2 - 2 q.k  via bf16x3-split 21-row contraction
         (exact bf16 products, fp32 accumulate; ~1e-7 accuracy)
  ACT  : sgn  = Sign(psum + (|q|^2-r^2))   (per-partition fp32 bias)
  DVE  : idx  = select(within & rank<=64, rank+C1, -1024)  (one fused custom
         op: compare + inclusive scan + mask; C1 = -1 / +63 for pair halves)
  Pool : out16[slot] = j via local_scatter over a 2-tile pair
  DVE  : pad empty (0) slots with the first neighbor; cast int32
"""

import numpy as np
from contextlib import ExitStack

RADIUS2 = float(np.float32(np.float32(0.1) ** 2))
B, N1, N2, K = 4, 2048, 8192, 64
NCORES = 8
QSHARD = N1 // 2  # 1024 queries per core

_CACHE = {}


# --------------------------------------------------------------------------
# custom DVE op registration
# --------------------------------------------------------------------------

def _register_ballq_op():
    import concourse.dve_ops as dvo
    from concourse.dve_spec import (
        Spec, Src0, Zero, C0, C1, C2, AluOp, scan, select, Bin, lower,
        _has_src1 as has_src1,
    )
    from concourse.dve_uop import DveOpSpec

    name = "BALLQ_IDX"
    if name in dvo._SUB_OPCODE_FOR_NAME:
        return (next(op for op in dvo.OPS if op.name == name),
                next(op for op in dvo.OPS if op.name == "BALLQ_IOTA"))

    w = Bin(AluOp.IS_LT, Src0, Zero)          # sgn < 0  -> within
    s = scan(AluOp.ADD, w)                    # inclusive rank among within
    body = select(w & (s <= C0), s + C1, C2)  # rank<=64 -> rank+C1 else -1024

    def _ref(in0, in1, c0, c1, c2):
        wn = in0 < 0
        sn = np.cumsum(wn, axis=1).astype(np.float32)
        return np.where(wn & (sn <= c0), sn + c1, c2).astype(np.float32)

    spec = Spec(body=body, reference=_ref)
    op = dvo.DveOp(name, spec, subdim=False, uops_sha={}, perf_en={"v3": True})
    dvo.OPS.append(op)
    dvo._SUB_OPCODE_FOR_NAME[name] = dvo._CUSTOM_DVE_ROW_BASE + len(dvo.OPS) - 1
    dvo.CUSTOM_DVE_SPECS[name] = spec
    from concourse.dve_spec import Idx
    spec2 = Spec(body=Idx + Src0 * Zero, reference=lambda in0, in1, c0, c1, c2:
                 np.broadcast_to(np.arange(in0.shape[1], dtype=np.float32),
                                 in0.shape).astype(np.float32))
    op2 = dvo.DveOp("BALLQ_IOTA", spec2, subdim=False, uops_sha={})
    dvo.OPS.append(op2)
    dvo._SUB_OPCODE_FOR_NAME["BALLQ_IOTA"] = dvo._CUSTOM_DVE_ROW_BASE + len(dvo.OPS) - 1
    dvo.CUSTOM_DVE_SPECS["BALLQ_IOTA"] = spec2
    for o, sp in ((op, spec), (op2, spec2)):
        for ver in ("v3", "v4"):
            try:
                compiled = DveOpSpec(
                    name=o.name,
                    opcode=dvo.get_dve_sub_opcode(o.name),
                    uops=lower(sp, ver=ver),
                    rd1_en=has_src1(sp),
                )
                o.uops_sha[ver] = compiled.sha(ver)
            except Exception:
                pass
    return op, op2


# --------------------------------------------------------------------------
# TileContext with the exit-drain wait-splitting workaround (this walrus
# build rejects sync waits attached to the CTRL drain instruction)
# --------------------------------------------------------------------------

def _make_tc_class():
    import concourse.tile as tile
    import concourse.mybir as mybir
    from concourse._compat import not_none as _nn
    from concourse.vector_clock import ScopedClock as _ScopedClock

    class SplitDrainTC(tile.TileContext):
        def _drain_and_barrier(self, tick_clock, wait_clock):
            nc = self.nc
            drain_inst = nc.sync.drain()
            wait_clock.add_sem_waits(
                drain_inst.ins, _ScopedClock({None: tick_clock.global_clock})
            )
            si = drain_inst.ins.sync_info
            if si is not None and si.on_wait:
                waits = list(si.on_wait)
                si.on_wait = []
                bb = _nn(nc.cur_bb).bb
                assert bb.instructions[-1] is drain_inst.ins
                bb.instructions.pop()
                for i in range(len(waits)):
                    nop = nc.sync.nop(hint="drain_wait", nofuse=True)
                    nop.ins.sync_info = mybir.SyncInfo(
                        on_wait=waits[i : i + 1], on_update=[]
                    )
                bb.instructions.append(drain_inst.ins)

            nc.all_engine_barrier()
            assert self.sems is not None
            popped = nc._tile_sem_poison_stack.pop()
            assert popped is self._sem_poison
            nc.clear_and_free_semaphores(list(self.sems.allocated().values()))
            nc.all_engine_barrier()

    return SplitDrainTC


# --------------------------------------------------------------------------
# the Bass program (SPMD: identical on all 8 cores)
# --------------------------------------------------------------------------

def _build_program():
    import concourse.bass as bass
    import concourse.bacc as bacc
    import concourse.mybir as mybir

    ballq_op, iota_op = _register_ballq_op()
    SplitDrainTC = _make_tc_class()
    f32 = mybir.dt.float32
    bf16 = mybir.dt.bfloat16
    i16 = mybir.dt.int16
    i32 = mybir.dt.int32

    nc = bacc.Bacc(None, target_bir_lowering=False)
    q_in = nc.declare_dram_parameter("q", [QSHARD, 3], f32, isOutput=False)
    k_in = nc.declare_dram_parameter("k", [N2, 3], f32, isOutput=False)
    out_t = nc.declare_dram_parameter("out", [QSHARD, K], i32, isOutput=True)

    # DRAM bounce for the 12 distinct key rows (bf16x3 splits + |k|^2 splits)
    kd = nc.dram_tensor("kd_bounce", [12, N2], bf16)

    ntiles = QSHARD // 128  # 8

    with SplitDrainTC(nc) as tc, ExitStack() as ctx:
        singles = ctx.enter_context(tc.tile_pool(name="singles", bufs=1))
        kprep = ctx.enter_context(tc.tile_pool(name="kprep", bufs=1))
        lhs_pool = ctx.enter_context(tc.tile_pool(name="lhs", bufs=1))
        qn_pool = ctx.enter_context(tc.tile_pool(name="qn", bufs=1))
        sgn_pool = ctx.enter_context(tc.tile_pool(name="sgn", bufs=2))
        idx_pool = ctx.enter_context(tc.tile_pool(name="idx", bufs=2))
        o16_pool = ctx.enter_context(tc.tile_pool(name="o16", bufs=3))
        fin_pool = ctx.enter_context(tc.tile_pool(name="fin", bufs=4))
        psum_pool = ctx.enter_context(tc.tile_pool(name="psum", bufs=2, space="PSUM"))
        qd_pool = ctx.enter_context(tc.tile_pool(name="qd", bufs=1, space="DRAM"))

        # ---- key prep: bf16x3 splits in natural layout, bounce to rows ----
        # knat[p, a*3+d] = k[64p + a, d]  (partition-major keys)
        knat = kprep.tile([128, 192], f32)
        nc.sync.dma_start(out=knat[:], in_=k_in[:, :].rearrange("(p a) d -> p (a d)", p=128))

        # planar split tiles: [128, 3, 64] (d-plane major) for contiguous bounce
        ka = kprep.tile([128, 192], bf16)
        kaV = ka[:].rearrange("p (d f) -> p f d", d=3)
        nc.vector.tensor_copy(kaV, knat[:].rearrange("p (f d) -> p f d", d=3))
        r1 = kprep.tile([128, 192], f32)
        nc.vector.tensor_sub(r1[:].rearrange("p (f d) -> p f d", d=3), knat[:].rearrange("p (f d) -> p f d", d=3), ka[:].rearrange("p (d f) -> p f d", d=3))
        kb = kprep.tile([128, 192], bf16)
        kbV = kb[:].rearrange("p (d f) -> p f d", d=3)
        nc.vector.tensor_copy(kbV, r1[:].rearrange("p (f d) -> p f d", d=3))
        r2 = kprep.tile([128, 192], f32)
        nc.vector.tensor_sub(r2[:].rearrange("p (f d) -> p f d", d=3), r1[:].rearrange("p (f d) -> p f d", d=3), kb[:].rearrange("p (d f) -> p f d", d=3))
        kc = kprep.tile([128, 192], bf16)
        kcV = kc[:].rearrange("p (d f) -> p f d", d=3)
        nc.vector.tensor_copy(kcV, r2[:].rearrange("p (f d) -> p f d", d=3))

        # |k|^2 (exact fp32 chain) and its bf16x3 split
        sq = kprep.tile([128, 192], f32)
        nc.vector.tensor_mul(sq[:], knat[:], knat[:])
        ksum = kprep.tile([128, 64], f32)
        nc.vector.tensor_reduce(
            ksum[:], sq[:].rearrange("p (a d) -> p a d", d=3),
            axis=mybir.AxisListType.X, op=mybir.AluOpType.add,
        )
        hA = kprep.tile([128, 64], bf16)
        nc.vector.tensor_copy(hA[:], ksum[:])
        hr1 = kprep.tile([128, 64], f32)
        nc.vector.tensor_sub(hr1[:], ksum[:], hA[:])
        hB = kprep.tile([128, 64], bf16)
        nc.vector.tensor_copy(hB[:], hr1[:])
        hr2 = kprep.tile([128, 64], f32)
        nc.vector.tensor_sub(hr2[:], hr1[:], hB[:])
        hC = kprep.tile([128, 64], bf16)
        nc.vector.tensor_copy(hC[:], hr2[:])

        # bounce out: kd rows 0-2=kaXYZ, 3-5=kbXYZ, 6-8=kcXYZ, 9-11=hABC
        # planar tiles: element (p, d, f) -> kd[d, 64p + f]; inner f contiguous
        for rows, t in ((0, ka), (3, kb), (6, kc)):
            nc.sync.dma_start(
                out=kd[rows:rows + 3, :].rearrange("d (p f) -> p d f", p=128),
                in_=t[:].rearrange("p (d f) -> p d f", d=3),
            )
        hAll = kprep.tile([128, 192], bf16)
        nc.vector.tensor_copy(hAll[:, 0:64], hA[:])
        nc.vector.tensor_copy(hAll[:, 64:128], hB[:])
        nc.vector.tensor_copy(hAll[:, 128:192], hC[:])
        nc.scalar.dma_start(
            out=kd[9:12, :].rearrange("d (p f) -> p d f", p=128),
            in_=hAll[:].rearrange("p (d f) -> p d f", d=3),
        )

        # bounce in: rhs rows (with duplicates) from kd
        # rhs: 0-2 ka, 3-5 ka, 6-8 ka, 9-11 kb, 12-14 kb, 15-17 kc, 18-20 h
        rhs = singles.tile([21, N2], bf16)
        for dst, src in ((0, 0), (3, 0), (6, 0), (9, 3), (12, 3), (15, 6), (18, 9)):
            nc.sync.dma_start(out=rhs[dst:dst + 3, :], in_=kd[src:src + 3, :])

        # ---- scatter data: values j then j again (pairs), via DVE Idx ----
        iota2 = singles.tile([128, 2 * N2], i16)
        nc.vector.memset(iota2[:], 0)
        for hf in range(2):
            nc.vector._custom_dve(
                iota_op,
                out=iota2[:, hf * N2:(hf + 1) * N2],
                in0=iota2[:, hf * N2:(hf + 1) * N2],
            )


        # ---- hoisted per-tile query prep (all tiles up front; keeps the
        # tiny DVE ops out of the scatter windows) -----------------------
        lhsTs, nbs = [], []
        for t in range(ntiles):
            qn = qn_pool.tile([128, 3], f32, tag=f"qn{t}")
            nc.sync.dma_start(out=qn[:], in_=q_in[t * 128:(t + 1) * 128, :])

            # bf16x3 split of q -> qall [128, 21]: the 21 lhsT rows as
            # columns, pre-scaled: [-2qa, -2qb, -2qc, -2qa, -2qb, -2qa, 1]
            qall = qn_pool.tile([128, 21], bf16, tag=f"qall{t}")
            nc.vector.tensor_copy(qall[:, 0:3], qn[:])
            qr1 = qn_pool.tile([128, 3], f32, tag=f"qr1{t}")
            nc.vector.tensor_sub(qr1[:], qn[:], qall[:, 0:3])
            nc.vector.tensor_copy(qall[:, 3:6], qr1[:])
            qr2 = qn_pool.tile([128, 3], f32, tag=f"qr2{t}")
            nc.vector.tensor_sub(qr2[:], qr1[:], qall[:, 3:6])
            nc.vector.tensor_copy(qall[:, 6:9], qr2[:])
            nc.vector.tensor_scalar_mul(qall[:, 9:12], qall[:, 0:3], 1.0)
            nc.vector.tensor_scalar_mul(qall[:, 12:15], qall[:, 3:6], 1.0)
            nc.vector.tensor_scalar_mul(qall[:, 15:18], qall[:, 0:3], 1.0)
            nc.vector.tensor_scalar_mul(qall[:, 0:18], qall[:, 0:18], -2.0)
            nc.vector.memset(qall[:, 18:21], 1.0)

            # bias nb = |q|^2 - r^2 (exact fp32 chain)
            qsq3 = qn_pool.tile([128, 3], f32, tag=f"qsq3{t}")
            nc.vector.tensor_mul(qsq3[:], qn[:], qn[:])
            nb = qn_pool.tile([128, 1], f32, tag=f"nb{t}")
            nc.vector.tensor_reduce(
                nb[:], qsq3[:], axis=mybir.AxisListType.X, op=mybir.AluOpType.add
            )
            nc.vector.tensor_scalar_add(nb[:], nb[:], -RADIUS2)
            nbs.append(nb)

            qd = qd_pool.tile([21, 128], bf16, tag=f"qd{t}")
            eng = nc.scalar if t % 2 == 0 else nc.sync
            eng.dma_start(out=qd[:].rearrange("r p -> p r"), in_=qall[:])
            lhsT = lhs_pool.tile([21, 128], bf16, tag=f"lhsT{t}")
            eng.dma_start(out=lhsT[:], in_=qd[:])
            lhsTs.append(lhsT)

        for pair in range(ntiles // 2):
            idx16 = idx_pool.tile([128, 2 * N2], i16)
            for half in range(2):
                t = pair * 2 + half
                lhsT = lhsTs[t]
                nb = nbs[t]

                # ---- matmuls + sign -------------------------------------
                sgn = sgn_pool.tile([128, N2], bf16, tag="sgn")
                for quarter in range(4):
                    psum = psum_pool.tile([128, 2048], f32, tag="psum")
                    for m in range(4):
                        c0 = quarter * 2048 + m * 512
                        nc.tensor.matmul(
                            psum[:, m * 512:(m + 1) * 512],
                            lhsT[:],
                            rhs[:, c0:c0 + 512],
                            start=True,
                            stop=True,
                        )
                    nc.scalar.activation(
                        out=sgn[:, quarter * 2048:(quarter + 1) * 2048],
                        in_=psum[:],
                        func=mybir.ActivationFunctionType.Sign,
                        bias=nb[:],
                        scale=1.0,
                    )

                # ---- fused compare+scan+mask -> int16 slots -------------
                # even half -> slots 0..63 (C1=-1); odd half -> 64..127 (C1=+63)
                nc.vector._custom_dve(
                    ballq_op,
                    out=idx16[:, half * N2:(half + 1) * N2],
                    in0=sgn[:],
                    s0=float(K),
                    s1=(-1.0 if half == 0 else 63.0),
                    imm2=-1024.0,
                )

            # ---- one scatter per pair of tiles --------------------------
            out16 = o16_pool.tile([128, 2 * K], i16)
            nc.gpsimd.local_scatter(
                out_ap=out16[:],
                data_ap=iota2[:],
                idxs_ap=idx16[:],
                channels=128,
                num_elems=2 * K,
                num_idxs=2 * N2,
            )

            # ---- pad + cast + store per half ----------------------------
            for half in range(2):
                t = pair * 2 + half
                sl = slice(half * K, (half + 1) * K)
                m01 = fin_pool.tile([128, K], mybir.dt.int8, tag="m01")
                nc.vector.tensor_scalar(
                    out=m01[:], in0=out16[:, sl], scalar1=0.0, scalar2=None,
                    op0=mybir.AluOpType.is_gt,
                )
                final = fin_pool.tile([128, K], i32, tag="final")
                nc.vector.select(
                    out=final[:],
                    mask=m01[:],
                    on_true=out16[:, sl],
                    on_false=out16[:, half * K:half * K + 1].to_broadcast([128, K]),
                )
                nc.sync.dma_start(out=out_t[t * 128:(t + 1) * 128, :], in_=final[:])

    nc.finalize()
    return nc


def _get_program():
    if "nc" not in _CACHE:
        _CACHE["nc"] = _build_program()
    return _CACHE["nc"]


# --------------------------------------------------------------------------
# public entry point
# --------------------------------------------------------------------------

def kernel(query: np.ndarray, key: np.ndarray) -> np.ndarray:
    from concourse.bass_utils import run_bass_kernel_spmd

    query = np.ascontiguousarray(np.asarray(query, dtype=np.float32))
    key = np.ascontiguousarray(np.asarray(key, dtype=np.float32))
    assert query.shape == (B, N1, 3) and key.shape == (B, N2, 3)

    nc = _get_program()

    in_maps = []
    for core in range(NCORES):
        b = core // 2
        h = core % 2
        in_maps.append({
            "q": np.ascontiguousarray(query[b, h * QSHARD:(h + 1) * QSHARD]),
            "k": np.ascontiguousarray(key[b]),
        })

    res = run_bass_kernel_spmd(nc, in_maps, core_ids=list(range(NCORES)))

    out = np.empty((B, N1, K), dtype=np.int32)
    for core in range(NCORES):
        b = core // 2
        h = core % 2
        out[b, h * QSHARD:(h + 1) * QSHARD] = res.results[core]["out"]
    return out



# revision 4
# speedup vs baseline: 1.6732x; 1.6732x over previous
"""Trainium2 Bass kernel for PointNet++-style ball query (nn_BallQuery).

Problem: query [4, 2048, 3] f32, key [4, 8192, 3] f32 -> out [4, 2048, 64] int32.
For each query point, the indices of the first 64 key points (in key order)
with squared distance < 0.1^2; empty slots padded with the first neighbor
index (0 if none).

Sharding (8 NeuronCores): data-parallel over batch B=4 (2 cores per batch),
queries split in halves of 1024 per core; keys of the batch replicated.

Per-core pipeline (8 tiles of 128 queries x 8192 keys, scatters paired):
  PE   : psum = |k|^2 - 2 q.k  via bf16x3-split 21-row contraction
         (exact bf16 products, fp32 accumulate; ~1e-7 accuracy).
         lhsT tiles produced by PE transposes against an inline identity.
  ACT  : sgn  = Sign(psum + (|q|^2-r^2))   (per-partition fp32 bias)
  DVE  : idx  = select(within & rank<=64, rank+C1, -1024)  (one fused custom
         op: compare + inclusive scan + mask; C1 = -1 / +63 for pair halves)
  Pool : out16[slot] = j via local_scatter over a 2-tile pair; scatter data
         (iota) is an inline NEFF constant.
  DVE  : pad empty (0) slots with the first neighbor; cast int32

Queries are partition-remapped (query q = 8p + a, a in 0..7 = tile index) so
the final store is a single contiguous-per-partition DMA.
"""

import numpy as np
from contextlib import ExitStack

RADIUS2 = float(np.float32(np.float32(0.1) ** 2))
B, N1, N2, K = 4, 2048, 8192, 64
NCORES = 8
QSHARD = N1 // 2  # 1024 queries per core
NT = QSHARD // 128  # 8 tiles (a-index)

_CACHE = {}


# --------------------------------------------------------------------------
# custom DVE op registration
# --------------------------------------------------------------------------

def _register_ballq_op():
    import concourse.dve_ops as dvo
    from concourse.dve_spec import (
        Spec, Src0, Zero, C0, C1, C2, AluOp, scan, select, Bin, lower,
        _has_src1 as has_src1,
    )
    from concourse.dve_uop import DveOpSpec

    name = "BALLQ_IDX"
    if name in dvo._SUB_OPCODE_FOR_NAME:
        return next(op for op in dvo.OPS if op.name == name)

    w = Bin(AluOp.IS_LT, Src0, Zero)          # sgn < 0  -> within
    s = scan(AluOp.ADD, w)                    # inclusive rank among within
    body = select(w & (s <= C0), s + C1, C2)  # rank<=64 -> rank+C1 else -1024

    def _ref(in0, in1, c0, c1, c2):
        wn = in0 < 0
        sn = np.cumsum(wn, axis=1).astype(np.float32)
        return np.where(wn & (sn <= c0), sn + c1, c2).astype(np.float32)

    spec = Spec(body=body, reference=_ref)
    op = dvo.DveOp(name, spec, subdim=False, uops_sha={}, perf_en={"v3": True})
    dvo.OPS.append(op)
    dvo._SUB_OPCODE_FOR_NAME[name] = dvo._CUSTOM_DVE_ROW_BASE + len(dvo.OPS) - 1
    dvo.CUSTOM_DVE_SPECS[name] = spec
    for ver in ("v3", "v4"):
        try:
            compiled = DveOpSpec(
                name=op.name,
                opcode=dvo.get_dve_sub_opcode(op.name),
                uops=lower(spec, ver=ver),
                rd1_en=has_src1(spec),
            )
            op.uops_sha[ver] = compiled.sha(ver)
        except Exception:
            pass
    return op


# --------------------------------------------------------------------------
# TileContext with the exit-drain wait-splitting workaround (this walrus
# build rejects sync waits attached to the CTRL drain instruction)
# --------------------------------------------------------------------------

def _make_tc_class():
    import concourse.tile as tile
    import concourse.mybir as mybir
    from concourse._compat import not_none as _nn
    from concourse.vector_clock import ScopedClock as _ScopedClock

    class SplitDrainTC(tile.TileContext):
        def _drain_and_barrier(self, tick_clock, wait_clock):
            nc = self.nc
            drain_inst = nc.sync.drain()
            wait_clock.add_sem_waits(
                drain_inst.ins, _ScopedClock({None: tick_clock.global_clock})
            )
            si = drain_inst.ins.sync_info
            if si is not None and si.on_wait:
                waits = list(si.on_wait)
                si.on_wait = []
                bb = _nn(nc.cur_bb).bb
                assert bb.instructions[-1] is drain_inst.ins
                bb.instructions.pop()
                for i in range(len(waits)):
                    nop = nc.sync.nop(hint="drain_wait", nofuse=True)
                    nop.ins.sync_info = mybir.SyncInfo(
                        on_wait=waits[i : i + 1], on_update=[]
                    )
                bb.instructions.append(drain_inst.ins)

            nc.all_engine_barrier()
            assert self.sems is not None
            popped = nc._tile_sem_poison_stack.pop()
            assert popped is self._sem_poison
            nc.clear_and_free_semaphores(list(self.sems.allocated().values()))
            nc.all_engine_barrier()

    return SplitDrainTC


# --------------------------------------------------------------------------
# the Bass program (SPMD: identical on all 8 cores)
# --------------------------------------------------------------------------

def _build_program():
    import ml_dtypes
    import concourse.bass as bass
    import concourse.bacc as bacc
    import concourse.mybir as mybir

    ballq_op = _register_ballq_op()
    SplitDrainTC = _make_tc_class()
    f32 = mybir.dt.float32
    bf16 = mybir.dt.bfloat16
    i16 = mybir.dt.int16
    i32 = mybir.dt.int32

    nc = bacc.Bacc(None, target_bir_lowering=False)
    q_in = nc.declare_dram_parameter("q", [QSHARD, 3], f32, isOutput=False)
    k_in = nc.declare_dram_parameter("k", [N2, 3], f32, isOutput=False)
    out_t = nc.declare_dram_parameter("out", [QSHARD, K], i32, isOutput=True)

    # inline constants baked into the NEFF
    iota_np = np.tile(np.arange(N2, dtype=np.int16), (128, 2))  # [128, 2*N2]
    iota_dram = nc.inline_tensor(iota_np, name="iota_c")
    ident_np = np.eye(128, dtype=ml_dtypes.bfloat16)
    ident_dram = nc.inline_tensor(ident_np, name="ident_c")

    with SplitDrainTC(nc) as tc, ExitStack() as ctx:
        singles = ctx.enter_context(tc.tile_pool(name="singles", bufs=1))
        kprep = ctx.enter_context(tc.tile_pool(name="kprep", bufs=1))
        qprep = ctx.enter_context(tc.tile_pool(name="qprep", bufs=1))
        lhs_pool = ctx.enter_context(tc.tile_pool(name="lhs", bufs=1))
        sgn_pool = ctx.enter_context(tc.tile_pool(name="sgn", bufs=2))
        idx_pool = ctx.enter_context(tc.tile_pool(name="idx", bufs=2))
        o16_pool = ctx.enter_context(tc.tile_pool(name="o16", bufs=3))
        fin_pool = ctx.enter_context(tc.tile_pool(name="fin", bufs=1))
        psum_pool = ctx.enter_context(tc.tile_pool(name="psum", bufs=3, space="PSUM"))
        tp_pool = ctx.enter_context(tc.tile_pool(name="tp", bufs=2, space="PSUM"))

        # ---- persistent SBUF constants ----------------------------------
        iota2 = singles.tile([128, 2 * N2], i16)
        nc.sync.dma_start(out=iota2[:], in_=iota_dram[:, :])
        ident = singles.tile([128, 128], bf16)
        nc.sync.dma_start(out=ident[:], in_=ident_dram[:, :])

        # ---- key prep: bf16x3 splits in natural layout ------------------
        # knat[p, a*3+d] = k[64p + a, d]  (partition-major keys)
        knat = kprep.tile([128, 192], f32)
        nc.sync.dma_start(out=knat[:], in_=k_in[:, :].rearrange("(p a) d -> p (a d)", p=128))

        # planar split tiles: [128, 3, 64] (d-plane major) for contiguous runs
        ka = kprep.tile([128, 192], bf16)
        kaV = ka[:].rearrange("p (d f) -> p f d", d=3)
        nc.vector.tensor_copy(kaV, knat[:].rearrange("p (f d) -> p f d", d=3))
        r1 = kprep.tile([128, 192], f32)
        nc.vector.tensor_sub(r1[:].rearrange("p (f d) -> p f d", d=3), knat[:].rearrange("p (f d) -> p f d", d=3), ka[:].rearrange("p (d f) -> p f d", d=3))
        kb = kprep.tile([128, 192], bf16)
        kbV = kb[:].rearrange("p (d f) -> p f d", d=3)
        nc.vector.tensor_copy(kbV, r1[:].rearrange("p (f d) -> p f d", d=3))
        r2 = kprep.tile([128, 192], f32)
        nc.vector.tensor_sub(r2[:].rearrange("p (f d) -> p f d", d=3), r1[:].rearrange("p (f d) -> p f d", d=3), kb[:].rearrange("p (d f) -> p f d", d=3))
        kc = kprep.tile([128, 192], bf16)
        kcV = kc[:].rearrange("p (d f) -> p f d", d=3)
        nc.vector.tensor_copy(kcV, r2[:].rearrange("p (f d) -> p f d", d=3))

        # |k|^2 (exact fp32 chain) and its bf16x3 split, planar-packed
        sq = kprep.tile([128, 192], f32)
        nc.vector.tensor_mul(sq[:], knat[:], knat[:])
        ksum = kprep.tile([128, 64], f32)
        nc.vector.tensor_reduce(
            ksum[:], sq[:].rearrange("p (a d) -> p a d", d=3),
            axis=mybir.AxisListType.X, op=mybir.AluOpType.add,
        )
        hAll = kprep.tile([128, 192], bf16)
        nc.vector.tensor_copy(hAll[:, 0:64], ksum[:])
        hr1 = kprep.tile([128, 64], f32)
        nc.vector.tensor_sub(hr1[:], ksum[:], hAll[:, 0:64])
        nc.vector.tensor_copy(hAll[:, 64:128], hr1[:])
        hr2 = kprep.tile([128, 64], f32)
        nc.vector.tensor_sub(hr2[:], hr1[:], hAll[:, 64:128])
        nc.vector.tensor_copy(hAll[:, 128:192], hr2[:])

        # bounce planar tiles through DRAM (SBUF->SBUF partition-role-swap
        # DMAs are illegal): kd rows 0-2=kaXYZ, 3-5=kbXYZ, 6-8=kcXYZ, 9-11=h.
        # Element (p, d, f) of a planar tile lands at kd[d, 64p + f].
        kd = nc.dram_tensor("kd_bounce", [12, N2], bf16)
        for rows, t in ((0, ka), (3, kb), (6, kc), (9, hAll)):
            nc.sync.dma_start(
                out=kd[rows:rows + 3, :].rearrange("d (p f) -> p d f", p=128),
                in_=t[:].rearrange("p (d f) -> p d f", d=3),
            )

        # bounce in: rhs rows (with duplicates) from kd
        # rhs: 0-2 ka, 3-5 ka, 6-8 ka, 9-11 kb, 12-14 kb, 15-17 kc, 18-20 h
        rhs = singles.tile([21, N2], bf16)
        for dst, src in ((0, 0), (3, 0), (6, 0), (9, 3), (12, 3), (15, 6), (18, 9)):
            nc.sync.dma_start(out=rhs[dst:dst + 3, :], in_=kd[src:src + 3, :])

        # ---- query prep (batched over all 8 tiles) ----------------------
        # qnat[p, a*3+d] = q[8p + a, d] : query index = 8p + a
        qnat = qprep.tile([128, 24], f32)
        nc.sync.dma_start(out=qnat[:], in_=q_in[:, :].rearrange("(p a) d -> p (a d)", p=128))

        # bf16x3 split of all queries
        qa = qprep.tile([128, 24], bf16)
        nc.vector.tensor_copy(qa[:], qnat[:])
        qr1 = qprep.tile([128, 24], f32)
        nc.vector.tensor_sub(qr1[:], qnat[:], qa[:])
        qb = qprep.tile([128, 24], bf16)
        nc.vector.tensor_copy(qb[:], qr1[:])
        qr2 = qprep.tile([128, 24], f32)
        nc.vector.tensor_sub(qr2[:], qr1[:], qb[:])
        qc = qprep.tile([128, 24], bf16)
        nc.vector.tensor_copy(qc[:], qr2[:])

        # qall[p, a, 0:21]: [-2qa, -2qb, -2qc, -2qa, -2qb, -2qa, 1,1,1]
        qall = qprep.tile([128, NT, 21], bf16)
        v3 = lambda t: t[:].rearrange("p (a d) -> p a d", d=3)
        for col, src in ((0, qa), (3, qb), (6, qc), (9, qa), (12, qb), (15, qa)):
            nc.vector.tensor_copy(qall[:, :, col:col + 3], v3(src))
        nc.vector.tensor_scalar_mul(qall[:, :, 0:18], qall[:, :, 0:18], -2.0)
        nc.vector.memset(qall[:, :, 18:21], 1.0)

        # bias nb[p, a] = |q|^2 - r^2 (exact fp32 chain)
        qsq = qprep.tile([128, 24], f32)
        nc.vector.tensor_mul(qsq[:], qnat[:], qnat[:])
        nball = qprep.tile([128, NT], f32)
        nc.vector.tensor_reduce(
            nball[:], qsq[:].rearrange("p (a d) -> p a d", d=3),
            axis=mybir.AxisListType.X, op=mybir.AluOpType.add,
        )
        nc.vector.tensor_scalar_add(nball[:], nball[:], -RADIUS2)

        # ---- PE transposes: qall[:, a, :] [128, 21] -> lhsT_a [21, 128] --
        lhsTs = []
        for a in range(NT):
            tp = tp_pool.tile([128, 1024], bf16, tag="tp")
            nc.tensor.matmul(
                tp[0:21, 0:128], qall[:, a, :], ident[:], is_transpose=True,
            )
            lhsT = lhs_pool.tile([21, 128], bf16, tag=f"lhsT{a}")
            nc.scalar.copy(lhsT[:], tp[0:21, 0:128])
            lhsTs.append(lhsT)

        # ---- main loop: pairs of tiles ----------------------------------
        fin = fin_pool.tile([128, NT * K], i32)
        for pair in range(NT // 2):
            idx16 = idx_pool.tile([128, 2 * N2], i16)
            for half in range(2):
                a = pair * 2 + half
                lhsT = lhsTs[a]

                # ---- matmuls + sign (psum eighths of 1024) --------------
                sgn = sgn_pool.tile([128, N2], bf16, tag="sgn")
                for e in range(8):
                    psum = psum_pool.tile([128, 1024], f32, tag="psum")
                    for m in range(2):
                        c0 = e * 1024 + m * 512
                        nc.tensor.matmul(
                            psum[:, m * 512:(m + 1) * 512],
                            lhsT[:],
                            rhs[:, c0:c0 + 512],
                            start=True,
                            stop=True,
                        )
                    nc.scalar.activation(
                        out=sgn[:, e * 1024:(e + 1) * 1024],
                        in_=psum[:],
                        func=mybir.ActivationFunctionType.Sign,
                        bias=nball[:, a:a + 1],
                        scale=1.0,
                    )

                # ---- fused compare+scan+mask -> int16 slots -------------
                # even half -> slots 0..63 (C1=-1); odd half -> 64..127
                nc.vector._custom_dve(
                    ballq_op,
                    out=idx16[:, half * N2:(half + 1) * N2],
                    in0=sgn[:],
                    s0=float(K),
                    s1=(-1.0 if half == 0 else 63.0),
                    imm2=-1024.0,
                )

            # ---- one scatter per pair of tiles --------------------------
            out16 = o16_pool.tile([128, 2 * K], i16)
            nc.gpsimd.local_scatter(
                out_ap=out16[:],
                data_ap=iota2[:],
                idxs_ap=idx16[:],
                channels=128,
                num_elems=2 * K,
                num_idxs=2 * N2,
            )

            # ---- pad + cast into fin slice per half ---------------------
            for half in range(2):
                a = pair * 2 + half
                sl = slice(half * K, (half + 1) * K)
                m01 = fin_pool.tile([128, K], mybir.dt.int8, tag="m01")
                nc.vector.tensor_scalar(
                    out=m01[:], in0=out16[:, sl], scalar1=0.0, scalar2=None,
                    op0=mybir.AluOpType.is_gt,
                )
                nc.vector.select(
                    out=fin[:, a * K:(a + 1) * K],
                    mask=m01[:],
                    on_true=out16[:, sl],
                    on_false=out16[:, half * K:half * K + 1].to_broadcast([128, K]),
                )

        # ---- single contiguous store: out row q = 8p + a ----------------
        nc.sync.dma_start(
            out=out_t[:, :].rearrange("(p a) k -> p (a k)", p=128),
            in_=fin[:],
        )

    nc.finalize()
    return nc


def _get_program():
    if "nc" not in _CACHE:
        _CACHE["nc"] = _build_program()
    return _CACHE["nc"]


# --------------------------------------------------------------------------
# public entry point
# --------------------------------------------------------------------------

def kernel(query: np.ndarray, key: np.ndarray) -> np.ndarray:
    from concourse.bass_utils import run_bass_kernel_spmd

    query = np.ascontiguousarray(np.asarray(query, dtype=np.float32))
    key = np.ascontiguousarray(np.asarray(key, dtype=np.float32))
    assert query.shape == (B, N1, 3) and key.shape == (B, N2, 3)

    nc = _get_program()

    in_maps = []
    for core in range(NCORES):
        b = core // 2
        h = core % 2
        in_maps.append({
            "q": np.ascontiguousarray(query[b, h * QSHARD:(h + 1) * QSHARD]),
            "k": np.ascontiguousarray(key[b]),
        })

    res = run_bass_kernel_spmd(nc, in_maps, core_ids=list(range(NCORES)))

    out = np.empty((B, N1, K), dtype=np.int32)
    for core in range(NCORES):
        b = core // 2
        h = core % 2
        out[b, h * QSHARD:(h + 1) * QSHARD] = res.results[core]["out"]
    return out


# revision 9
# speedup vs baseline: 1.7704x; 1.0581x over previous
"""Trainium2 Bass kernel for PointNet++-style ball query (nn_BallQuery).

Problem: query [4, 2048, 3] f32, key [4, 8192, 3] f32 -> out [4, 2048, 64] int32.
For each query point, the indices of the first 64 key points (in key order)
with squared distance < 0.1^2; empty slots padded with the first neighbor
index (0 if none).

Sharding (8 NeuronCores): data-parallel over batch B=4 (2 cores per batch),
queries split in halves of 1024 per core; keys of the batch replicated.

Per-core pipeline (8 tiles of 128 queries x 8192 keys, scatters paired):
  PE   : psum = |k|^2 - 2 q.k  via bf16x3-split 21-row contraction
         (exact bf16 products, fp32 accumulate; ~1e-7 accuracy).
         lhsT tiles produced by PE transposes against an inline identity.
  ACT  : sgn  = Sign(psum + (|q|^2-r^2))   (per-partition fp32 bias)
  DVE  : idx  = select(within & rank<=64, rank+C1, -1024)  (one fused custom
         op: compare + inclusive scan + mask; C1 = -1 / +63 for pair halves)
  Pool : out16[slot] = j via local_scatter over a 2-tile pair; scatter data
         (iota) is an inline NEFF constant.
  DVE  : pad empty (0) slots with the first neighbor; cast int32

Queries are partition-remapped (query q = 8p + a, a in 0..7 = tile index) so
the final store is a single contiguous-per-partition DMA.
"""

import numpy as np
from contextlib import ExitStack

RADIUS2 = float(np.float32(np.float32(0.1) ** 2))
B, N1, N2, K = 4, 2048, 8192, 64
NCORES = 8
QSHARD = N1 // 2  # 1024 queries per core
NT = QSHARD // 128  # 8 tiles (a-index)

_CACHE = {}


# --------------------------------------------------------------------------
# custom DVE op registration
# --------------------------------------------------------------------------

def _register_ballq_op():
    import concourse.dve_ops as dvo
    from concourse.dve_spec import (
        Spec, Src0, Zero, C0, C1, C2, AluOp, scan, select, Bin, lower,
        _has_src1 as has_src1,
    )
    from concourse.dve_uop import DveOpSpec

    name = "BALLQ_IDX"
    if name in dvo._SUB_OPCODE_FOR_NAME:
        return next(op for op in dvo.OPS if op.name == name)

    w = Bin(AluOp.IS_LT, Src0, Zero)          # sgn < 0  -> within
    s = scan(AluOp.ADD, w)                    # inclusive rank among within
    body = select(w & (s <= C0), s + C1, C2)  # rank<=64 -> rank+C1 else -1024

    def _ref(in0, in1, c0, c1, c2):
        wn = in0 < 0
        sn = np.cumsum(wn, axis=1).astype(np.float32)
        return np.where(wn & (sn <= c0), sn + c1, c2).astype(np.float32)

    spec = Spec(body=body, reference=_ref)
    op = dvo.DveOp(name, spec, subdim=False, uops_sha={}, perf_en={"v3": True})
    dvo.OPS.append(op)
    dvo._SUB_OPCODE_FOR_NAME[name] = dvo._CUSTOM_DVE_ROW_BASE + len(dvo.OPS) - 1
    dvo.CUSTOM_DVE_SPECS[name] = spec
    for ver in ("v3", "v4"):
        try:
            compiled = DveOpSpec(
                name=op.name,
                opcode=dvo.get_dve_sub_opcode(op.name),
                uops=lower(spec, ver=ver),
                rd1_en=has_src1(spec),
            )
            op.uops_sha[ver] = compiled.sha(ver)
        except Exception:
            pass
    return op


# --------------------------------------------------------------------------
# TileContext with the exit-drain wait-splitting workaround (this walrus
# build rejects sync waits attached to the CTRL drain instruction)
# --------------------------------------------------------------------------

def _make_tc_class():
    import concourse.tile as tile
    import concourse.mybir as mybir
    from concourse._compat import not_none as _nn
    from concourse.vector_clock import ScopedClock as _ScopedClock

    class SplitDrainTC(tile.TileContext):
        def _drain_and_barrier(self, tick_clock, wait_clock):
            nc = self.nc
            drain_inst = nc.sync.drain()
            wait_clock.add_sem_waits(
                drain_inst.ins, _ScopedClock({None: tick_clock.global_clock})
            )
            si = drain_inst.ins.sync_info
            if si is not None and si.on_wait:
                waits = list(si.on_wait)
                si.on_wait = []
                bb = _nn(nc.cur_bb).bb
                assert bb.instructions[-1] is drain_inst.ins
                bb.instructions.pop()
                for i in range(len(waits)):
                    nop = nc.sync.nop(hint="drain_wait", nofuse=True)
                    nop.ins.sync_info = mybir.SyncInfo(
                        on_wait=waits[i : i + 1], on_update=[]
                    )
                bb.instructions.append(drain_inst.ins)

            nc.all_engine_barrier()
            assert self.sems is not None
            popped = nc._tile_sem_poison_stack.pop()
            assert popped is self._sem_poison
            nc.clear_and_free_semaphores(list(self.sems.allocated().values()))
            nc.all_engine_barrier()

    return SplitDrainTC


# --------------------------------------------------------------------------
# the Bass program (SPMD: identical on all 8 cores)
# --------------------------------------------------------------------------

def _build_program():
    import ml_dtypes
    import concourse.bass as bass
    import concourse.bacc as bacc
    import concourse.mybir as mybir

    ballq_op = _register_ballq_op()
    SplitDrainTC = _make_tc_class()
    f32 = mybir.dt.float32
    bf16 = mybir.dt.bfloat16
    i16 = mybir.dt.int16
    i32 = mybir.dt.int32

    nc = bacc.Bacc(None, target_bir_lowering=False)
    q_in = nc.declare_dram_parameter("q", [QSHARD, 3], f32, isOutput=False)
    k_in = nc.declare_dram_parameter("k", [N2, 3], f32, isOutput=False)
    out_t = nc.declare_dram_parameter("out", [QSHARD, K], i32, isOutput=True)

    # inline constants baked into the NEFF
    iota_np = np.tile(np.arange(N2, dtype=np.int16), (128, 1))  # [128, N2]
    iota_dram = nc.inline_tensor(iota_np, name="iota_c")
    ident_np = np.eye(128, dtype=ml_dtypes.bfloat16)
    ident_dram = nc.inline_tensor(ident_np, name="ident_c")

    with SplitDrainTC(nc) as tc, ExitStack() as ctx:
        singles = ctx.enter_context(tc.tile_pool(name="singles", bufs=1))
        kprep = ctx.enter_context(tc.tile_pool(name="kprep", bufs=1))
        qprep = ctx.enter_context(tc.tile_pool(name="qprep", bufs=1))
        lhs_pool = ctx.enter_context(tc.tile_pool(name="lhs", bufs=1))
        sgn_pool = ctx.enter_context(tc.tile_pool(name="sgn", bufs=2))
        idx_pool = ctx.enter_context(tc.tile_pool(name="idx", bufs=2))
        o16_pool = ctx.enter_context(tc.tile_pool(name="o16", bufs=1))
        fin_pool = ctx.enter_context(tc.tile_pool(name="fin", bufs=1))
        psum_pool = ctx.enter_context(tc.tile_pool(name="psum", bufs=3, space="PSUM"))
        tp_pool = ctx.enter_context(tc.tile_pool(name="tp", bufs=2, space="PSUM"))

        # ---- input loads first (the fat iota const load goes last so it
        # doesn't queue ahead of the latency-critical key/query loads) ----
        knat = kprep.tile([128, 192], f32)
        nc.sync.dma_start(out=knat[:], in_=k_in[:, :].rearrange("(p a) d -> p (a d)", p=128))
        qnat = qprep.tile([128, 24], f32)
        nc.sync.dma_start(out=qnat[:], in_=q_in[:, :].rearrange("(p a) d -> p (a d)", p=128))
        ident = singles.tile([128, 128], bf16)
        nc.sync.dma_start(out=ident[:], in_=ident_dram[:, :])
        iota2 = singles.tile([128, N2], i16)
        nc.scalar.dma_start(out=iota2[:], in_=iota_dram[:, :])

        # ---- key prep: bf16x3 splits in natural layout ------------------
        # knat[p, a*3+d] = k[64p + a, d]  (partition-major keys)

        # planar split tiles: [128, 3, 64] (d-plane major) for contiguous runs
        ka = kprep.tile([128, 192], bf16)
        kaV = ka[:].rearrange("p (d f) -> p f d", d=3)
        nc.vector.tensor_copy(kaV, knat[:].rearrange("p (f d) -> p f d", d=3))
        r1 = kprep.tile([128, 192], f32)
        nc.vector.tensor_sub(r1[:].rearrange("p (f d) -> p f d", d=3), knat[:].rearrange("p (f d) -> p f d", d=3), ka[:].rearrange("p (d f) -> p f d", d=3))
        kb = kprep.tile([128, 192], bf16)
        kbV = kb[:].rearrange("p (d f) -> p f d", d=3)
        nc.vector.tensor_copy(kbV, r1[:].rearrange("p (f d) -> p f d", d=3))
        r2 = kprep.tile([128, 192], f32)
        nc.vector.tensor_sub(r2[:].rearrange("p (f d) -> p f d", d=3), r1[:].rearrange("p (f d) -> p f d", d=3), kb[:].rearrange("p (d f) -> p f d", d=3))
        kc = kprep.tile([128, 192], bf16)
        kcV = kc[:].rearrange("p (d f) -> p f d", d=3)
        nc.vector.tensor_copy(kcV, r2[:].rearrange("p (f d) -> p f d", d=3))

        # |k|^2 (exact fp32 chain) and its bf16x3 split, planar-packed
        sq = kprep.tile([128, 192], f32)
        nc.vector.tensor_mul(sq[:], knat[:], knat[:])
        ksum = kprep.tile([128, 64], f32)
        nc.vector.tensor_reduce(
            ksum[:], sq[:].rearrange("p (a d) -> p a d", d=3),
            axis=mybir.AxisListType.X, op=mybir.AluOpType.add,
        )
        hAll = kprep.tile([128, 192], bf16)
        nc.vector.tensor_copy(hAll[:, 0:64], ksum[:])
        hr1 = kprep.tile([128, 64], f32)
        nc.vector.tensor_sub(hr1[:], ksum[:], hAll[:, 0:64])
        nc.vector.tensor_copy(hAll[:, 64:128], hr1[:])
        hr2 = kprep.tile([128, 64], f32)
        nc.vector.tensor_sub(hr2[:], hr1[:], hAll[:, 64:128])
        nc.vector.tensor_copy(hAll[:, 128:192], hr2[:])

        # bounce planar tiles through DRAM (SBUF->SBUF partition-role-swap
        # DMAs are illegal): kd rows 0-2=kaXYZ, 3-5=kbXYZ, 6-8=kcXYZ, 9-11=h.
        # Element (p, d, f) of a planar tile lands at kd[d, 64p + f].
        kd = nc.dram_tensor("kd_bounce", [12, N2], bf16)
        for rows, t in ((0, ka), (3, kb), (6, kc), (9, hAll)):
            nc.sync.dma_start(
                out=kd[rows:rows + 3, :].rearrange("d (p f) -> p d f", p=128),
                in_=t[:].rearrange("p (d f) -> p d f", d=3),
            )

        # bounce in: rhs rows (with duplicates) from kd
        # rhs: 0-2 ka, 3-5 ka, 6-8 ka, 9-11 kb, 12-14 kb, 15-17 kc, 18-20 h
        rhs = singles.tile([21, N2], bf16)
        for dst, src in ((0, 0), (3, 0), (6, 0), (9, 3), (12, 3), (15, 6), (18, 9)):
            nc.sync.dma_start(out=rhs[dst:dst + 3, :], in_=kd[src:src + 3, :])

        # ---- query prep (batched over all 8 tiles) ----------------------
        # qnat[p, a*3+d] = q[8p + a, d] : query index = 8p + a
        # bf16x3 split of all queries
        qa = qprep.tile([128, 24], bf16)
        nc.vector.tensor_copy(qa[:], qnat[:])
        qr1 = qprep.tile([128, 24], f32)
        nc.vector.tensor_sub(qr1[:], qnat[:], qa[:])
        qb = qprep.tile([128, 24], bf16)
        nc.vector.tensor_copy(qb[:], qr1[:])
        qr2 = qprep.tile([128, 24], f32)
        nc.vector.tensor_sub(qr2[:], qr1[:], qb[:])
        qc = qprep.tile([128, 24], bf16)
        nc.vector.tensor_copy(qc[:], qr2[:])

        # qall[p, a, 0:21]: [-2qa, -2qb, -2qc, -2qa, -2qb, -2qa, 1,1,1]
        qall = qprep.tile([128, NT, 21], bf16)
        v3 = lambda t: t[:].rearrange("p (a d) -> p a d", d=3)
        for col, src in ((0, qa), (3, qb), (6, qc), (9, qa), (12, qb), (15, qa)):
            nc.vector.tensor_copy(qall[:, :, col:col + 3], v3(src))
        nc.vector.tensor_scalar_mul(qall[:, :, 0:18], qall[:, :, 0:18], -2.0)
        nc.vector.memset(qall[:, :, 18:21], 1.0)

        # bias nb[p, a] = |q|^2 - r^2 (exact fp32 chain)
        qsq = qprep.tile([128, 24], f32)
        nc.vector.tensor_mul(qsq[:], qnat[:], qnat[:])
        nball = qprep.tile([128, NT], f32)
        nc.vector.tensor_reduce(
            nball[:], qsq[:].rearrange("p (a d) -> p a d", d=3),
            axis=mybir.AxisListType.X, op=mybir.AluOpType.add,
        )
        nc.vector.tensor_scalar_add(nball[:], nball[:], -RADIUS2)

        # ---- PE transposes: qall[:, a, :] [128, 21] -> lhsT_a [21, 128] --
        lhsTs = []
        for a in range(NT):
            tp = tp_pool.tile([128, 1024], bf16, tag="tp")
            nc.tensor.matmul(
                tp[0:21, 0:128], qall[:, a, :], ident[:], is_transpose=True,
            )
            lhsT = lhs_pool.tile([21, 128], bf16, tag=f"lhsT{a}")
            nc.scalar.copy(lhsT[:], tp[0:21, 0:128])
            lhsTs.append(lhsT)

        # ---- main loop: one scatter per tile; pads deferred by 2 tiles so
        # they never block later scans in the in-order DVE queue -----------
        fin = fin_pool.tile([128, NT * K], i32)
        out16s = []

        def emit_pad(a):
            out16 = out16s[a]
            m01 = fin_pool.tile([128, K], mybir.dt.int8, tag="m01")
            nc.vector.tensor_scalar(
                out=m01[:], in0=out16[:], scalar1=0.0, scalar2=None,
                op0=mybir.AluOpType.is_gt,
            )
            nc.vector.select(
                out=fin[:, a * K:(a + 1) * K],
                mask=m01[:],
                on_true=out16[:],
                on_false=out16[:, 0:1].to_broadcast([128, K]),
            )

        for a in range(NT):
            lhsT = lhsTs[a]

            # ---- matmuls + sign (psum eighths of 1024) ------------------
            sgn = sgn_pool.tile([128, N2], bf16, tag="sgn")
            for e in range(8):
                psum = psum_pool.tile([128, 1024], f32, tag="psum")
                for m in range(2):
                    c0 = e * 1024 + m * 512
                    nc.tensor.matmul(
                        psum[:, m * 512:(m + 1) * 512],
                        lhsT[:],
                        rhs[:, c0:c0 + 512],
                        start=True,
                        stop=True,
                    )
                nc.scalar.activation(
                    out=sgn[:, e * 1024:(e + 1) * 1024],
                    in_=psum[:],
                    func=mybir.ActivationFunctionType.Sign,
                    bias=nball[:, a:a + 1],
                    scale=1.0,
                )

            # ---- fused compare+scan+mask -> int16 slots -----------------
            idx16 = idx_pool.tile([128, N2], i16)
            nc.vector._custom_dve(
                ballq_op,
                out=idx16[:],
                in0=sgn[:],
                s0=float(K),
                s1=-1.0,
                imm2=-1024.0,
            )

            # ---- scatter ------------------------------------------------
            out16 = o16_pool.tile([128, K], i16, tag=f"o16_{a % 4}")
            nc.gpsimd.local_scatter(
                out_ap=out16[:],
                data_ap=iota2[:],
                idxs_ap=idx16[:],
                channels=128,
                num_elems=K,
                num_idxs=N2,
            )
            out16s.append(out16)

            if a >= 2:
                emit_pad(a - 2)
        emit_pad(NT - 2)
        emit_pad(NT - 1)

        # ---- single contiguous store: out row q = 8p + a ----------------
        nc.sync.dma_start(
            out=out_t[:, :].rearrange("(p a) k -> p (a k)", p=128),
            in_=fin[:],
        )

    nc.finalize()
    return nc


def _get_program():
    if "nc" not in _CACHE:
        _CACHE["nc"] = _build_program()
    return _CACHE["nc"]


# --------------------------------------------------------------------------
# public entry point
# --------------------------------------------------------------------------

def kernel(query: np.ndarray, key: np.ndarray) -> np.ndarray:
    from concourse.bass_utils import run_bass_kernel_spmd

    query = np.ascontiguousarray(np.asarray(query, dtype=np.float32))
    key = np.ascontiguousarray(np.asarray(key, dtype=np.float32))
    assert query.shape == (B, N1, 3) and key.shape == (B, N2, 3)

    nc = _get_program()

    in_maps = []
    for core in range(NCORES):
        b = core // 2
        h = core % 2
        in_maps.append({
            "q": np.ascontiguousarray(query[b, h * QSHARD:(h + 1) * QSHARD]),
            "k": np.ascontiguousarray(key[b]),
        })

    res = run_bass_kernel_spmd(nc, in_maps, core_ids=list(range(NCORES)))

    out = np.empty((B, N1, K), dtype=np.int32)
    for core in range(NCORES):
        b = core // 2
        h = core % 2
        out[b, h * QSHARD:(h + 1) * QSHARD] = res.results[core]["out"]
    return out


# revision 13
# speedup vs baseline: 1.9940x; 1.1263x over previous
"""Trainium2 Bass kernel for PointNet++-style ball query (nn_BallQuery).

Problem: query [4, 2048, 3] f32, key [4, 8192, 3] f32 -> out [4, 2048, 64] int32.
For each query point, the indices of the first 64 key points (in key order)
with squared distance < 0.1^2; empty slots padded with the first neighbor
index (0 if none).

Sharding (8 NeuronCores): data-parallel over batch B=4 (2 cores per batch),
queries split in halves of 1024 per core; keys of the batch replicated.

Per-core pipeline (8 tiles of 128 queries x 8192 keys, scatters paired):
  PE   : psum = |k|^2 - 2 q.k  via bf16x3-split 21-row contraction
         (exact bf16 products, fp32 accumulate; ~1e-7 accuracy).
         lhsT tiles produced by PE transposes against an inline identity.
  ACT  : sgn  = Sign(psum + (|q|^2-r^2))   (per-partition fp32 bias)
  DVE  : idx  = select(within & rank<=64, rank+C1, -1024)  (one fused custom
         op: compare + inclusive scan + mask; C1 = -1 / +63 for pair halves)
  Pool : out16[slot] = j via local_scatter over a 2-tile pair; scatter data
         (iota) is an inline NEFF constant.
  DVE  : pad empty (0) slots with the first neighbor; cast int32

Queries are partition-remapped (query q = 8p + a, a in 0..7 = tile index) so
the final store is a single contiguous-per-partition DMA.
"""

import numpy as np
from contextlib import ExitStack

RADIUS2 = float(np.float32(np.float32(0.1) ** 2))
B, N1, N2, K = 4, 2048, 8192, 64
NCORES = 8
QSHARD = N1 // 2  # 1024 queries per core
NT = QSHARD // 128  # 8 tiles (a-index)

_CACHE = {}


# --------------------------------------------------------------------------
# custom DVE op registration
# --------------------------------------------------------------------------

def _register_ballq_op():
    import concourse.dve_ops as dvo
    from concourse.dve_spec import (
        Spec, Src0, Zero, C0, C1, C2, AluOp, scan, select, Bin, lower,
        _has_src1 as has_src1,
    )
    from concourse.dve_uop import DveOpSpec

    name = "BALLQ_IDX"
    if name in dvo._SUB_OPCODE_FOR_NAME:
        return next(op for op in dvo.OPS if op.name == name)

    w = Bin(AluOp.IS_LT, Src0, Zero)          # sgn < 0  -> within
    s = scan(AluOp.ADD, w)                    # inclusive rank among within
    body = select(w & (s <= C0), s + C1, C2)  # rank<=64 -> rank+C1 else -1024

    def _ref(in0, in1, c0, c1, c2):
        wn = in0 < 0
        sn = np.cumsum(wn, axis=1).astype(np.float32)
        return np.where(wn & (sn <= c0), sn + c1, c2).astype(np.float32)

    spec = Spec(body=body, reference=_ref)
    op = dvo.DveOp(name, spec, subdim=False, uops_sha={}, perf_en={"v3": True})
    dvo.OPS.append(op)
    dvo._SUB_OPCODE_FOR_NAME[name] = dvo._CUSTOM_DVE_ROW_BASE + len(dvo.OPS) - 1
    dvo.CUSTOM_DVE_SPECS[name] = spec
    for ver in ("v3", "v4"):
        try:
            compiled = DveOpSpec(
                name=op.name,
                opcode=dvo.get_dve_sub_opcode(op.name),
                uops=lower(spec, ver=ver),
                rd1_en=has_src1(spec),
            )
            op.uops_sha[ver] = compiled.sha(ver)
        except Exception:
            pass
    return op


# --------------------------------------------------------------------------
# TileContext with the exit-drain wait-splitting workaround (this walrus
# build rejects sync waits attached to the CTRL drain instruction)
# --------------------------------------------------------------------------

def _make_tc_class():
    import concourse.tile as tile
    import concourse.mybir as mybir
    from concourse._compat import not_none as _nn
    from concourse.vector_clock import ScopedClock as _ScopedClock

    class SplitDrainTC(tile.TileContext):
        def _drain_and_barrier(self, tick_clock, wait_clock):
            nc = self.nc
            drain_inst = nc.sync.drain()
            wait_clock.add_sem_waits(
                drain_inst.ins, _ScopedClock({None: tick_clock.global_clock})
            )
            si = drain_inst.ins.sync_info
            if si is not None and si.on_wait:
                waits = list(si.on_wait)
                si.on_wait = []
                bb = _nn(nc.cur_bb).bb
                assert bb.instructions[-1] is drain_inst.ins
                bb.instructions.pop()
                for i in range(len(waits)):
                    nop = nc.sync.nop(hint="drain_wait", nofuse=True)
                    nop.ins.sync_info = mybir.SyncInfo(
                        on_wait=waits[i : i + 1], on_update=[]
                    )
                bb.instructions.append(drain_inst.ins)

            nc.all_engine_barrier()
            assert self.sems is not None
            popped = nc._tile_sem_poison_stack.pop()
            assert popped is self._sem_poison
            nc.clear_and_free_semaphores(list(self.sems.allocated().values()))
            nc.all_engine_barrier()

    return SplitDrainTC


# --------------------------------------------------------------------------
# the Bass program (SPMD: identical on all 8 cores)
# --------------------------------------------------------------------------

def _build_program():
    import ml_dtypes
    import concourse.bass as bass
    import concourse.bacc as bacc
    import concourse.mybir as mybir

    ballq_op = _register_ballq_op()
    SplitDrainTC = _make_tc_class()
    f32 = mybir.dt.float32
    bf16 = mybir.dt.bfloat16
    i16 = mybir.dt.int16
    i32 = mybir.dt.int32

    nc = bacc.Bacc(None, target_bir_lowering=False)
    q_in = nc.declare_dram_parameter("q", [QSHARD, 3], f32, isOutput=False)
    k_in = nc.declare_dram_parameter("k", [N2, 3], f32, isOutput=False)
    out_t = nc.declare_dram_parameter("out", [QSHARD, K], i32, isOutput=True)

    # inline constants baked into the NEFF
    iota_np = np.tile(np.arange(N2, dtype=np.int16), (128, 1))  # [128, N2]
    iota_dram = nc.inline_tensor(iota_np, name="iota_c")
    ident_np = np.eye(128, dtype=ml_dtypes.bfloat16)
    ident_dram = nc.inline_tensor(ident_np, name="ident_c")

    with SplitDrainTC(nc) as tc, ExitStack() as ctx:
        singles = ctx.enter_context(tc.tile_pool(name="singles", bufs=1))
        kprep = ctx.enter_context(tc.tile_pool(name="kprep", bufs=1))
        qprep = ctx.enter_context(tc.tile_pool(name="qprep", bufs=1))
        lhs_pool = ctx.enter_context(tc.tile_pool(name="lhs", bufs=1))
        sgn_pool = ctx.enter_context(tc.tile_pool(name="sgn", bufs=2))
        idx_pool = ctx.enter_context(tc.tile_pool(name="idx", bufs=2))
        o16_pool = ctx.enter_context(tc.tile_pool(name="o16", bufs=1))
        fin_pool = ctx.enter_context(tc.tile_pool(name="fin", bufs=1))
        psum_pool = ctx.enter_context(tc.tile_pool(name="psum", bufs=3, space="PSUM"))
        tp_pool = ctx.enter_context(tc.tile_pool(name="tp", bufs=2, space="PSUM"))

        # ---- input loads first (the fat iota const load goes last so it
        # doesn't queue ahead of the latency-critical key/query loads; the
        # tensor/vector sequencers reach their first dma_start soonest) ----
        knat = kprep.tile([128, 192], f32)
        nc.gpsimd.dma_start(out=knat[:], in_=k_in[:, :].rearrange("(p a) d -> p (a d)", p=128))
        qnat = qprep.tile([128, 24], f32)
        nc.gpsimd.dma_start(out=qnat[:], in_=q_in[:, :].rearrange("(p a) d -> p (a d)", p=128))
        ident = singles.tile([128, 128], bf16)
        nc.scalar.dma_start(out=ident[:], in_=ident_dram[:, :])
        iota2 = singles.tile([128, N2], i16)
        nc.scalar.dma_start(out=iota2[:], in_=iota_dram[:, :])

        # ---- key prep: bf16x3 splits in natural layout ------------------
        # knat[p, a*3+d] = k[64p + a, d]  (partition-major keys)

        # planar split tiles: [128, 3, 64] (d-plane major) for contiguous runs
        ka = kprep.tile([128, 192], bf16)
        kaV = ka[:].rearrange("p (d f) -> p f d", d=3)
        nc.vector.tensor_copy(kaV, knat[:].rearrange("p (f d) -> p f d", d=3))
        r1 = kprep.tile([128, 192], f32)
        nc.vector.tensor_sub(r1[:].rearrange("p (f d) -> p f d", d=3), knat[:].rearrange("p (f d) -> p f d", d=3), ka[:].rearrange("p (d f) -> p f d", d=3))
        kb = kprep.tile([128, 192], bf16)
        kbV = kb[:].rearrange("p (d f) -> p f d", d=3)
        nc.vector.tensor_copy(kbV, r1[:].rearrange("p (f d) -> p f d", d=3))
        r2 = kprep.tile([128, 192], f32)
        nc.vector.tensor_sub(r2[:].rearrange("p (f d) -> p f d", d=3), r1[:].rearrange("p (f d) -> p f d", d=3), kb[:].rearrange("p (d f) -> p f d", d=3))
        kc = kprep.tile([128, 192], bf16)
        kcV = kc[:].rearrange("p (d f) -> p f d", d=3)
        nc.vector.tensor_copy(kcV, r2[:].rearrange("p (f d) -> p f d", d=3))

        # |k|^2 (exact fp32 chain) and its bf16x3 split, planar-packed
        sq = kprep.tile([128, 192], f32)
        nc.vector.tensor_mul(sq[:], knat[:], knat[:])
        ksum = kprep.tile([128, 64], f32)
        nc.vector.tensor_reduce(
            ksum[:], sq[:].rearrange("p (a d) -> p a d", d=3),
            axis=mybir.AxisListType.X, op=mybir.AluOpType.add,
        )
        hAll = kprep.tile([128, 192], bf16)
        nc.vector.tensor_copy(hAll[:, 0:64], ksum[:])
        hr1 = kprep.tile([128, 64], f32)
        nc.vector.tensor_sub(hr1[:], ksum[:], hAll[:, 0:64])
        nc.vector.tensor_copy(hAll[:, 64:128], hr1[:])
        hr2 = kprep.tile([128, 64], f32)
        nc.vector.tensor_sub(hr2[:], hr1[:], hAll[:, 64:128])
        nc.vector.tensor_copy(hAll[:, 128:192], hr2[:])

        # bounce planar tiles through DRAM (SBUF->SBUF partition-role-swap
        # DMAs are illegal): kd rows 0-2=kaXYZ, 3-5=kbXYZ, 6-8=kcXYZ, 9-11=h.
        # Element (p, d, f) of a planar tile lands at kd[d, 64p + f].
        kd = nc.dram_tensor("kd_bounce", [12, N2], bf16)
        for rows, t in ((0, ka), (3, kb), (6, kc), (9, hAll)):
            nc.sync.dma_start(
                out=kd[rows:rows + 3, :].rearrange("d (p f) -> p d f", p=128),
                in_=t[:].rearrange("p (d f) -> p d f", d=3),
            )

        # bounce in: rhs rows (with duplicates) from kd
        # rhs: 0-2 ka, 3-5 ka, 6-8 ka, 9-11 kb, 12-14 kb, 15-17 kc, 18-20 h
        rhs = singles.tile([21, N2], bf16)
        for dst, src in ((0, 0), (3, 0), (6, 0), (9, 3), (12, 3), (15, 6), (18, 9)):
            nc.sync.dma_start(out=rhs[dst:dst + 3, :], in_=kd[src:src + 3, :])

        # ---- query prep (batched over all 8 tiles) ----------------------
        # qnat[p, a*3+d] = q[8p + a, d] : query index = 8p + a
        # bf16x3 split of all queries
        qa = qprep.tile([128, 24], bf16)
        nc.vector.tensor_copy(qa[:], qnat[:])
        qr1 = qprep.tile([128, 24], f32)
        nc.vector.tensor_sub(qr1[:], qnat[:], qa[:])
        qb = qprep.tile([128, 24], bf16)
        nc.vector.tensor_copy(qb[:], qr1[:])
        qr2 = qprep.tile([128, 24], f32)
        nc.vector.tensor_sub(qr2[:], qr1[:], qb[:])
        qc = qprep.tile([128, 24], bf16)
        nc.vector.tensor_copy(qc[:], qr2[:])

        # qall[p, a, 0:21]: [-2qa, -2qb, -2qc, -2qa, -2qb, -2qa, 1,1,1]
        qall = qprep.tile([128, NT, 21], bf16)
        v3 = lambda t: t[:].rearrange("p (a d) -> p a d", d=3)
        for col, src in ((0, qa), (3, qb), (6, qc), (9, qa), (12, qb), (15, qa)):
            nc.vector.tensor_copy(qall[:, :, col:col + 3], v3(src))
        nc.vector.tensor_scalar_mul(qall[:, :, 0:18], qall[:, :, 0:18], -2.0)
        nc.vector.memset(qall[:, :, 18:21], 1.0)

        # bias nb[p, a] = |q|^2 - r^2 (exact fp32 chain)
        qsq = qprep.tile([128, 24], f32)
        nc.vector.tensor_mul(qsq[:], qnat[:], qnat[:])
        nball = qprep.tile([128, NT], f32)
        nc.vector.tensor_reduce(
            nball[:], qsq[:].rearrange("p (a d) -> p a d", d=3),
            axis=mybir.AxisListType.X, op=mybir.AluOpType.add,
        )
        nc.vector.tensor_scalar_add(nball[:], nball[:], -RADIUS2)

        # ---- PE transposes: qall[:, a, :] [128, 21] -> lhsT_a [21, 128] --
        lhsTs = []
        for a in range(NT):
            tp = tp_pool.tile([128, 1024], bf16, tag="tp")
            nc.tensor.matmul(
                tp[0:21, 0:128], qall[:, a, :], ident[:], is_transpose=True,
            )
            lhsT = lhs_pool.tile([21, 128], bf16, tag=f"lhsT{a}")
            nc.scalar.copy(lhsT[:], tp[0:21, 0:128])
            lhsTs.append(lhsT)

        # ---- main loop: one scatter per tile; pads deferred by 2 tiles so
        # they never block later scans in the in-order DVE queue -----------
        fin = fin_pool.tile([128, NT * K], i32)
        out16s = []

        def emit_pad(a):
            out16 = out16s[a]
            m01 = fin_pool.tile([128, K], mybir.dt.int8, tag="m01")
            nc.vector.tensor_scalar(
                out=m01[:], in0=out16[:], scalar1=0.0, scalar2=None,
                op0=mybir.AluOpType.is_gt,
            )
            nc.vector.select(
                out=fin[:, a * K:(a + 1) * K],
                mask=m01[:],
                on_true=out16[:],
                on_false=out16[:, 0:1].to_broadcast([128, K]),
            )

        for a in range(NT):
            lhsT = lhsTs[a]

            # ---- matmuls + sign (psum eighths of 1024) ------------------
            sgn = sgn_pool.tile([128, N2], bf16, tag="sgn")
            for e in range(8):
                psum = psum_pool.tile([128, 1024], f32, tag="psum")
                for m in range(2):
                    c0 = e * 1024 + m * 512
                    nc.tensor.matmul(
                        psum[:, m * 512:(m + 1) * 512],
                        lhsT[:],
                        rhs[:, c0:c0 + 512],
                        start=True,
                        stop=True,
                    )
                nc.scalar.activation(
                    out=sgn[:, e * 1024:(e + 1) * 1024],
                    in_=psum[:],
                    func=mybir.ActivationFunctionType.Sign,
                    bias=nball[:, a:a + 1],
                    scale=1.0,
                )

            # ---- fused compare+scan+mask -> int16 slots -----------------
            idx16 = idx_pool.tile([128, N2], i16)
            nc.vector._custom_dve(
                ballq_op,
                out=idx16[:],
                in0=sgn[:],
                s0=float(K),
                s1=-1.0,
                imm2=-1024.0,
            )

            # ---- scatter ------------------------------------------------
            out16 = o16_pool.tile([128, K], i16, tag=f"o16_{a}")
            nc.gpsimd.local_scatter(
                out_ap=out16[:],
                data_ap=iota2[:],
                idxs_ap=idx16[:],
                channels=128,
                num_elems=K,
                num_idxs=N2,
            )
            out16s.append(out16)

        # pads all emitted after the last scatter: in the conservative
        # vector-clock model each pad waits on the newest scatter emitted
        # before it, so interleaving them would stall the in-order DVE queue
        for a in range(NT):
            emit_pad(a)

        # ---- single contiguous store: out row q = 8p + a ----------------
        nc.sync.dma_start(
            out=out_t[:, :].rearrange("(p a) k -> p (a k)", p=128),
            in_=fin[:],
        )

    nc.finalize()
    return nc


def _get_program():
    if "nc" not in _CACHE:
        _CACHE["nc"] = _build_program()
    return _CACHE["nc"]


# --------------------------------------------------------------------------
# public entry point
# --------------------------------------------------------------------------

def kernel(query: np.ndarray, key: np.ndarray) -> np.ndarray:
    from concourse.bass_utils import run_bass_kernel_spmd

    query = np.ascontiguousarray(np.asarray(query, dtype=np.float32))
    key = np.ascontiguousarray(np.asarray(key, dtype=np.float32))
    assert query.shape == (B, N1, 3) and key.shape == (B, N2, 3)

    nc = _get_program()

    in_maps = []
    for core in range(NCORES):
        b = core // 2
        h = core % 2
        in_maps.append({
            "q": np.ascontiguousarray(query[b, h * QSHARD:(h + 1) * QSHARD]),
            "k": np.ascontiguousarray(key[b]),
        })

    res = run_bass_kernel_spmd(nc, in_maps, core_ids=list(range(NCORES)))

    out = np.empty((B, N1, K), dtype=np.int32)
    for core in range(NCORES):
        b = core // 2
        h = core % 2
        out[b, h * QSHARD:(h + 1) * QSHARD] = res.results[core]["out"]
    return out
